# revision 1
# baseline (speedup 1.0000x reference)
"""HGNN layer kernel for 8 TRN2 NeuronCores (Bass/Tile, SPMD row-sharded).

Math (reference):
    dv = H.sum(1); de = H.sum(0)
    Xs = X * dv^-1/2
    M  = H^T @ Xs            [E, F]
    M  = M * de^-1
    Xn = (H @ M) * dv^-1/2   [N, F]
    out = Xn @ W^T + b

Distribution: rows of X/H sharded over 8 cores (N=8192 -> 1024 rows/core).
GEMM1 (H^T @ Xs) is a local partial GEMM; the [E, F] partial plus the
partial column-sum row `de` are fused into ONE AllReduce of [F+1, E].
Everything after that is row-parallel.

Layout trick: GEMM1 is computed transposed (M^T = Xs^T-as-stationary, H
moving) so the AllReduce buffer is [F+1, E] with partition=f. Post-AR,
M'^T chunks [fi,128e] serve as matmul *stationary* operands against the
moving W^T, which lands Mw in [e(part), fo] layout directly -- no on-chip
transposes anywhere (H^T comes pre-transposed from the host shard prep).
"""

import os
import sys
import types

import numpy as np


def _ensure_axon_hooks_module():
    """bass_utils imports antenv.axon_hooks when tracing; some images
    lack it. Provide a stub (and try to wire the real ctypes hook) so
    trace paths degrade gracefully instead of crashing."""
    try:
        import antenv.axon_hooks  # noqa: F401
        return
    except ImportError:
        pass
    try:
        import antenv
    except ImportError:
        return
    mod = types.ModuleType("antenv.axon_hooks")
    state = {"hook": None}
    mod.get_axon_ntff_profile_hook = lambda: state["hook"]
    mod.set_axon_ntff_profile_hook = lambda h: state.__setitem__("hook", h)
    sys.modules["antenv.axon_hooks"] = mod
    antenv.axon_hooks = mod
    try:
        from trn_agent_boot.trn_boot import _ntff_profile_via_ctypes
        hook = _ntff_profile_via_ctypes("/opt/axon/libaxon_pjrt.so")
        if hook is not None:
            state["hook"] = hook
    except Exception:
        pass


_ensure_axon_hooks_module()

N, E, F = 8192, 1024, 256
P = 128
NC_COUNT = 8
NL = N // NC_COUNT          # 1024 rows per core
NT = NL // P                # 8 row tiles per core
ET = E // P                 # 8 e-chunks
FI = F // P                 # 2 fi-chunks

# matmul compute dtype: "f32r" (full-rate, ~tf32 precision), "f32" (1/4 rate,
# full precision). H/ones stay exact in either mode.
MM_DTYPE = os.environ.get("HGNN_MM_DTYPE", "f32r")

_cache = {}


def _build():
    from concourse import bacc, bass, tile, mybir

    f32 = mybir.dt.float32

    nc = bacc.Bacc("TRN2", target_bir_lowering=False, debug=False,
                   num_devices=NC_COUNT)

    X_d = nc.dram_tensor("X", [NL, F], f32, kind="ExternalInput")
    H_d = nc.dram_tensor("H", [NL, E], f32, kind="ExternalInput")
    HT_d = nc.dram_tensor("HT", [E, NL], f32, kind="ExternalInput")
    WT_d = nc.dram_tensor("WT", [F, F], f32, kind="ExternalInput")
    B_d = nc.dram_tensor("bias", [P, F], f32, kind="ExternalInput")
    ONES_d = nc.dram_tensor("ones", [P, 1], f32, kind="ExternalInput")
    out_d = nc.dram_tensor("out", [NL, F], f32, kind="ExternalOutput")

    if MM_DTYPE == "f32r":
        R = mybir.dt.float32r

        def rc(ap):
            return ap.bitcast(R)
    else:
        R = f32

        def rc(ap):
            return ap

    with tile.TileContext(nc) as tc:
        with (
            tc.tile_pool(name="const", bufs=1) as constp,
            tc.tile_pool(name="hp", bufs=1) as hp,
            tc.tile_pool(name="htp", bufs=1) as htp,
            tc.tile_pool(name="xp", bufs=1) as xp,
            tc.tile_pool(name="sp", bufs=1) as sp,
            tc.tile_pool(name="mtout", bufs=4) as mtoutp,
            tc.tile_pool(name="mwp", bufs=1) as mwp,
            tc.tile_pool(name="outp", bufs=3) as outp,
            tc.tile_pool(name="ps_mt", bufs=2, space="PSUM") as ps_mt,
            tc.tile_pool(name="ps_de", bufs=2, space="PSUM") as ps_de,
            tc.tile_pool(name="ps_b", bufs=3, space="PSUM") as ps_b,
            tc.tile_pool(name="dram", bufs=1, space="DRAM") as dramp,
        ):
            # ---- ones first (gates the de matmuls at the head of the PE
            # stream), then H on the sync queue while X/consts go via gpsimd.
            ones = constp.tile([P, 1], R)
            nc.sync.dma_start(ones[:], rc(ONES_d[:, :]))

            h = []
            for i in range(NT):
                hi = hp.tile([P, E], R, name=f"h{i}")
                nc.sync.dma_start(hi[:], rc(H_d[i * P:(i + 1) * P, :]))
                h.append(hi)

            x = []
            for i in range(NT):
                xi = xp.tile([P, F], f32, name=f"x{i}")
                nc.gpsimd.dma_start(xi[:], X_d[i * P:(i + 1) * P, :])
                x.append(xi)

            wt = []
            for c in range(FI):
                wtc = constp.tile([P, F], R, name=f"wt{c}")
                nc.gpsimd.dma_start(wtc[:], rc(WT_d[c * P:(c + 1) * P, :]))
                wt.append(wtc)
            bias = constp.tile([P, F], f32)
            nc.gpsimd.dma_start(bias[:], B_d[:, :])

            # dv chain (per tile): DVE rowsum -> DVE recip -> ACT sqrt -> DVE mul
            xs, dvis = [], []
            for i in range(NT):
                dv = sp.tile([P, 1], f32, name=f"dv{i}")
                nc.vector.tensor_reduce(dv[:], h[i][:].bitcast(f32),
                                        mybir.AxisListType.X,
                                        mybir.AluOpType.add)
                dvr = sp.tile([P, 1], f32, name=f"dvr{i}")
                nc.vector.reciprocal(dvr[:], dv[:])
                dvi = sp.tile([P, 1], f32, name=f"dvis{i}")
                nc.scalar.sqrt(dvi[:], dvr[:])
                dvis.append(dvi)

                xsi = xp.tile([P, F], R, name=f"xs{i}")
                nc.vector.tensor_scalar_mul(xsi[:], x[i][:], dvi[:])
                xs.append(xsi)

            # ---- collective bounce buffers ----
            cc_in = dramp.tile([F + 1, E], f32, name="cc_in")
            cc_out = dramp.tile([F + 1, E], f32, name="cc_out",
                                addr_space="Shared")

            # ---- de row first: de[e] = sum_n H[n, e] (needs only H, so the
            # PE computes it while the dv/xs chain is still running) ----
            EH = 512  # moving free-dim per matmul
            for eh in range(E // EH):
                de_ps = ps_de.tile([1, EH], f32, name="de_ps")
                for i in range(NT):
                    nc.tensor.matmul(
                        de_ps[:], ones[:],
                        h[i][:, eh * EH:(eh + 1) * EH],
                        start=(i == 0), stop=(i == NT - 1),
                    )
                de_sb = mtoutp.tile([1, EH], f32, name="de_sb")
                nc.scalar.copy(de_sb[:], de_ps[:])
                nc.sync.dma_start(cc_in[F:F + 1, eh * EH:(eh + 1) * EH],
                                  de_sb[:])

            # ---- GEMM1: M^T[f, e] = sum_n Xs[n, f] * H[n, e] ----
            for jf in range(FI):
                for eh in range(E // EH):
                    mt_ps = ps_mt.tile([P, EH], f32, name="mt_ps")
                    for i in range(NT):
                        nc.tensor.matmul(
                            mt_ps[:],
                            xs[i][:, jf * P:(jf + 1) * P],
                            h[i][:, eh * EH:(eh + 1) * EH],
                            start=(i == 0), stop=(i == NT - 1),
                        )
                    mt_sb = mtoutp.tile([P, EH], f32, name="mt_sb")
                    nc.vector.tensor_copy(mt_sb[:], mt_ps[:])
                    nc.sync.dma_start(
                        cc_in[jf * P:(jf + 1) * P, eh * EH:(eh + 1) * EH],
                        mt_sb[:])

            # ---- AllReduce of [M^T | de] over all 8 cores ----
            nc.gpsimd.collective_compute(
                "AllReduce",
                mybir.AluOpType.add,
                replica_groups=[list(range(NC_COUNT))],
                ins=[cc_in[:].opt()],
                outs=[cc_out[:].opt()],
            )

            # ---- H^T tiles (host-pretransposed); overlap with AllReduce ----
            ht = []
            for j in range(ET):
                htj = htp.tile([P, NL], R, name=f"ht{j}")
                nc.gpsimd.dma_start(htj[:], rc(HT_d[j * P:(j + 1) * P, :]))
                ht.append(htj)

            # ---- read back: M'^T fi-chunks + de (reshaped to [128, 8]) ----
            mtin = []
            for c in range(FI):
                mc = mwp.tile([P, E], R, name=f"mtin{c}")
                nc.sync.dma_start(mc[:], rc(cc_out[c * P:(c + 1) * P, :]))
                mtin.append(mc)
            de_sb2 = sp.tile([P, ET], f32)
            nc.sync.dma_start(
                de_sb2[:],
                cc_out[F:F + 1, :].rearrange("o (c p) -> (o p) c", p=P))
            de_inv = sp.tile([P, ET], f32)
            nc.vector.reciprocal(de_inv[:], de_sb2[:])

            # ---- GEMM-W: Mw[e, fo] = sum_fi M'[e, fi] W^T[fi, fo]; x de^-1 ----
            mw = []
            for j in range(ET):
                mw_ps = ps_b.tile([P, F], f32, name="mw_ps", tag="ps_post")
                for c in range(FI):
                    nc.tensor.matmul(
                        mw_ps[:],
                        mtin[c][:, j * P:(j + 1) * P],
                        wt[c][:],
                        start=(c == 0), stop=(c == FI - 1),
                    )
                mwj = mwp.tile([P, F], R, name=f"mw{j}")
                nc.vector.tensor_scalar_mul(mwj[:], mw_ps[:],
                                            de_inv[:, j:j + 1])
                mw.append(mwj)

            # ---- GEMM2: out[n, fo] = (sum_e H^T[e,n] Mw[e,fo]) * dv^-1/2 + b ----
            for jn in range(NT):
                o_ps = ps_b.tile([P, F], f32, name="o_ps", tag="ps_post")
                for j in range(ET):
                    nc.tensor.matmul(
                        o_ps[:],
                        ht[j][:, jn * P:(jn + 1) * P],
                        mw[j][:],
                        start=(j == 0), stop=(j == ET - 1),
                    )
                ot = outp.tile([P, F], f32, name="ot")
                nc.vector.scalar_tensor_tensor(
                    ot[:], o_ps[:], dvis[jn][:], bias[:],
                    op0=mybir.AluOpType.mult, op1=mybir.AluOpType.add)
                nc.sync.dma_start(out_d[jn * P:(jn + 1) * P, :], ot[:])

    nc.compile()
    return nc


def _get_nc():
    if "nc" not in _cache:
        _cache["nc"] = _build()
    return _cache["nc"]


def kernel(X, H, W, b):
    from concourse import bass_utils

    nc = _get_nc()

    X = np.asarray(X, dtype=np.float32)
    H = np.asarray(H, dtype=np.float32)
    W = np.asarray(W, dtype=np.float32)
    b = np.asarray(b, dtype=np.float32)

    WT = np.ascontiguousarray(W.T)
    bias = np.ascontiguousarray(np.tile(b[None, :], (P, 1)))
    ones_col = np.ones((P, 1), dtype=np.float32)

    in_maps = []
    for c in range(NC_COUNT):
        sl = slice(c * NL, (c + 1) * NL)
        Hc = np.ascontiguousarray(H[sl])
        in_maps.append({
            "X": np.ascontiguousarray(X[sl]),
            "H": Hc,
            "HT": np.ascontiguousarray(Hc.T),
            "WT": WT,
            "bias": bias,
            "ones": ones_col,
        })

    res = bass_utils.run_bass_kernel_spmd(
        nc, in_maps, core_ids=list(range(NC_COUNT)),
        trace=bool(int(os.environ.get("HGNN_TRACE", "0"))),
    )
    _cache["last_result"] = res
    out = np.concatenate([res.results[c]["out"] for c in range(NC_COUNT)],
                         axis=0)
    return out



# revision 3
# speedup vs baseline: 1.1619x; 1.1619x over previous
"""HGNN layer kernel for 8 TRN2 NeuronCores (Bass/Tile, SPMD row-sharded).

Math (reference):
    dv = H.sum(1); de = H.sum(0)
    out = Dv^-1/2 H De^-1 H^T Dv^-1/2 X W^T + b

Host folds the diagonal scalings into H once:
    Hs = Dv^-1/2 H          (rows scaled)
    A  = Hs De^-1           (columns scaled)
    out = A @ (Hs^T X W^T) + b

Distribution: rows of X/Hs/A sharded over 8 cores (N=8192 -> 1024/core).
Device pipeline per core (all matmul operands fp16, PSUM f32):
    GEMM1: M^T[f, e]   = sum_n X[n, f] Hs[n, e]     (local partial)
    GEMMW: mw[e, fo]   = sum_fi M^T[fi, e] W^T[fi, fo]   (pre-AR, linearity)
    AllReduce over mw [E, F] in fp16, chunked so transfer overlaps compute
    GEMM2: out^T[f, n] = sum_e mw[e, f] A^T[e, n] (+ bias)
Output is stored transposed [F, NL]; the host transposes back.
"""

import os
import sys
import types

import numpy as np


def _ensure_axon_hooks_module():
    """bass_utils imports antenv.axon_hooks when tracing; some images
    lack it. Provide a stub (and try to wire the real ctypes hook) so
    trace paths degrade gracefully instead of crashing."""
    try:
        import antenv.axon_hooks  # noqa: F401
        return
    except ImportError:
        pass
    try:
        import antenv
    except ImportError:
        return
    mod = types.ModuleType("antenv.axon_hooks")
    state = {"hook": None}
    mod.get_axon_ntff_profile_hook = lambda: state["hook"]
    mod.set_axon_ntff_profile_hook = lambda h: state.__setitem__("hook", h)
    sys.modules["antenv.axon_hooks"] = mod
    antenv.axon_hooks = mod
    try:
        from trn_agent_boot.trn_boot import _ntff_profile_via_ctypes
        hook = _ntff_profile_via_ctypes("/opt/axon/libaxon_pjrt.so")
        if hook is not None:
            state["hook"] = hook
    except Exception:
        pass


_ensure_axon_hooks_module()

N, E, F = 8192, 1024, 256
P = 128
NC_COUNT = 8
NL = N // NC_COUNT          # 1024 rows per core
NT = NL // P                # 8 row tiles per core
ET = E // P                 # 8 e-chunks of 128
FI = F // P                 # 2 fi-chunks
EH = 512                    # e-half width (one PSUM bank of f32)
NH = 512                    # n-half width for GEMM2 psums

# number of AllReduce chunks the [E, F] mw tensor is split into
AR_CHUNKS = int(os.environ.get("HGNN_AR_CHUNKS", "2"))
assert ET % AR_CHUNKS == 0

_cache = {}


def _build():
    from concourse import bacc, bass, tile, mybir

    f32 = mybir.dt.float32
    f16 = mybir.dt.float16

    nc = bacc.Bacc("TRN2", target_bir_lowering=False, debug=False,
                   num_devices=NC_COUNT)

    X_d = nc.dram_tensor("X", [NL, F], f16, kind="ExternalInput")
    HS_d = nc.dram_tensor("HS", [NL, E], f16, kind="ExternalInput")
    AT_d = nc.dram_tensor("AT", [E, NL], f16, kind="ExternalInput")
    WT_d = nc.dram_tensor("WT", [F, F], f16, kind="ExternalInput")
    B_d = nc.dram_tensor("bias", [F, 1], f32, kind="ExternalInput")
    out_d = nc.dram_tensor("out", [F, NL], f32, kind="ExternalOutput")

    ecpc = ET // AR_CHUNKS          # 128-row e-chunks per AR chunk

    with tile.TileContext(nc) as tc:
        with (
            tc.tile_pool(name="const", bufs=1) as constp,
            tc.tile_pool(name="xp", bufs=1) as xp,
            tc.tile_pool(name="hsp", bufs=1) as hsp,
            tc.tile_pool(name="atp", bufs=1) as atp,
            tc.tile_pool(name="mtp", bufs=4) as mtp,
            tc.tile_pool(name="mwp", bufs=4) as mwp,
            tc.tile_pool(name="mrp", bufs=1) as mrp,
            tc.tile_pool(name="outp", bufs=3) as outp,
            tc.tile_pool(name="ps_mt", bufs=2, space="PSUM") as ps_mt,
            tc.tile_pool(name="ps_mw", bufs=2, space="PSUM") as ps_mw,
            tc.tile_pool(name="ps_o", bufs=1, space="PSUM") as ps_o,
            tc.tile_pool(name="dram", bufs=1, space="DRAM") as dramp,
        ):
            # ---- constants on the scalar queue ----
            wt = []
            for c in range(FI):
                wtc = constp.tile([P, F], f16, name=f"wt{c}")
                nc.scalar.dma_start(wtc[:], WT_d[c * P:(c + 1) * P, :])
                wt.append(wtc)
            bias = []
            for c in range(FI):
                bc = constp.tile([P, 1], f32, name=f"bias{c}")
                nc.scalar.dma_start(bc[:], B_d[c * P:(c + 1) * P, :])
                bias.append(bc)

            # ---- X and Hs(half 0) interleaved on the sync queue so GEMM1
            # can start after the first pair lands ----
            x = [xp.tile([P, F], f16, name=f"x{i}") for i in range(NT)]
            hs = [[hsp.tile([P, EH], f16, name=f"hs{h}_{i}")
                   for i in range(NT)] for h in range(2)]
            for i in range(NT):
                nc.sync.dma_start(x[i][:], X_d[i * P:(i + 1) * P, :])
                nc.sync.dma_start(hs[0][i][:], HS_d[i * P:(i + 1) * P, 0:EH])
            for i in range(NT):
                nc.sync.dma_start(hs[1][i][:], HS_d[i * P:(i + 1) * P, EH:E])

            # ---- A^T tiles for GEMM2 (needed only post-AR) ----
            at = []
            for j in range(ET):
                atj = atp.tile([P, NL], f16, name=f"at{j}")
                nc.scalar.dma_start(atj[:], AT_d[j * P:(j + 1) * P, :])
                at.append(atj)

            # ---- collective bounce buffers, one pair per AR chunk ----
            cc_in, cc_out = [], []
            for k in range(AR_CHUNKS):
                cc_in.append(dramp.tile([ecpc * P, F], f16, name=f"cc_in{k}"))
                cc_out.append(dramp.tile([ecpc * P, F], f16, name=f"cc_out{k}",
                                         addr_space="Shared"))

            # ---- GEMM1 + GEMMW per e-half; AR chunk fires when its rows
            # are written ----
            for half in range(2):
                mt_sb = []
                for fi in range(FI):
                    ps = ps_mt.tile([P, EH], f32, name="mt_ps")
                    for i in range(NT):
                        nc.tensor.matmul(
                            ps[:], x[i][:, fi * P:(fi + 1) * P],
                            hs[half][i][:],
                            start=(i == 0), stop=(i == NT - 1),
                        )
                    sb = mtp.tile([P, EH], f16, name="mt_sb")
                    if fi == 0:
                        nc.vector.tensor_copy(sb[:], ps[:])
                    else:
                        nc.scalar.copy(sb[:], ps[:])
                    mt_sb.append(sb)
                for jj in range(ET // 2):        # e-chunks within this half
                    j = half * (ET // 2) + jj    # global e-chunk index
                    psw = ps_mw.tile([P, F], f32, name="mw_ps")
                    for fi in range(FI):
                        nc.tensor.matmul(
                            psw[:], mt_sb[fi][:, jj * P:(jj + 1) * P],
                            wt[fi][:],
                            start=(fi == 0), stop=(fi == FI - 1),
                        )
                    mws = mwp.tile([P, F], f16, name="mw_sb")
                    if jj % 2 == 0:
                        nc.vector.tensor_copy(mws[:], psw[:])
                    else:
                        nc.scalar.copy(mws[:], psw[:])
                    k, r = divmod(j, ecpc)
                    nc.sync.dma_start(cc_in[k][r * P:(r + 1) * P, :], mws[:])
                    if r == ecpc - 1:
                        nc.gpsimd.collective_compute(
                            "AllReduce",
                            mybir.AluOpType.add,
                            replica_groups=[list(range(NC_COUNT))],
                            ins=[cc_in[k][:].opt()],
                            outs=[cc_out[k][:].opt()],
                        )

            # ---- read back reduced mw chunks; GEMM2 accumulates them into
            # four persistent out^T psums as they arrive ----
            mwr = []
            for j in range(ET):
                k, r = divmod(j, ecpc)
                t = mrp.tile([P, F], f16, name=f"mwr{j}")
                nc.sync.dma_start(t[:], cc_out[k][r * P:(r + 1) * P, :])
                mwr.append(t)

            pso = [[ps_o.tile([P, NH], f32, name=f"o_ps{f}{nh}")
                    for nh in range(2)] for f in range(FI)]
            for j in range(ET):
                for f in range(FI):
                    for nh in range(2):
                        nc.tensor.matmul(
                            pso[f][nh][:], mwr[j][:, f * P:(f + 1) * P],
                            at[j][:, nh * NH:(nh + 1) * NH],
                            start=(j == 0), stop=(j == ET - 1),
                        )
            for f in range(FI):
                for nh in range(2):
                    ot = outp.tile([P, NH], f32, name="ot")
                    nc.vector.tensor_scalar_add(ot[:], pso[f][nh][:],
                                                bias[f][:])
                    nc.sync.dma_start(
                        out_d[f * P:(f + 1) * P, nh * NH:(nh + 1) * NH],
                        ot[:])

    nc.compile()
    return nc


def _get_nc():
    if "nc" not in _cache:
        _cache["nc"] = _build()
    return _cache["nc"]


def kernel(X, H, W, b):
    from concourse import bass_utils

    nc = _get_nc()

    X = np.asarray(X, dtype=np.float32)
    H = np.asarray(H, dtype=np.float32)
    W = np.asarray(W, dtype=np.float32)
    b = np.asarray(b, dtype=np.float32)

    dv = H.sum(axis=1)
    de = H.sum(axis=0)
    dvis = (1.0 / np.sqrt(dv)).astype(np.float32)
    dei = (1.0 / de).astype(np.float32)

    Hs32 = H * dvis[:, None]
    HS16 = Hs32.astype(np.float16)
    A16 = (Hs32 * dei[None, :]).astype(np.float16)
    X16 = X.astype(np.float16)
    WT16 = np.ascontiguousarray(W.T).astype(np.float16)
    bias_col = np.ascontiguousarray(b[:, None]).astype(np.float32)

    in_maps = []
    for c in range(NC_COUNT):
        sl = slice(c * NL, (c + 1) * NL)
        in_maps.append({
            "X": np.ascontiguousarray(X16[sl]),
            "HS": np.ascontiguousarray(HS16[sl]),
            "AT": np.ascontiguousarray(A16[sl].T),
            "WT": WT16,
            "bias": bias_col,
        })

    res = bass_utils.run_bass_kernel_spmd(
        nc, in_maps, core_ids=list(range(NC_COUNT)),
        trace=bool(int(os.environ.get("HGNN_TRACE", "0"))),
    )
    _cache["last_result"] = res
    out = np.concatenate(
        [res.results[c]["out"].T for c in range(NC_COUNT)], axis=0)
    return np.ascontiguousarray(out, dtype=np.float32)


# revision 6
# speedup vs baseline: 1.4659x; 1.2616x over previous
"""HGNN layer kernel for 8 TRN2 NeuronCores (Bass/Tile, SPMD row-sharded).

Math (reference):
    dv = H.sum(1); de = H.sum(0)
    out = Dv^-1/2 H De^-1 H^T Dv^-1/2 X W^T + b

Host folds the diagonal scalings into H once:
    Hs = Dv^-1/2 H          (rows scaled)
    A  = Hs De^-1           (columns scaled)
    out = A @ (Hs^T X W^T) + b

Distribution: rows of X/Hs/A sharded over 8 cores (N=8192 -> 1024/core).
Device pipeline per core (fp16 operands, f32 PSUM):
    GEMM1: M^T[f, e]   = sum_n X[n, f] Hs[n, e]          (local partial)
    GEMMW: mw[e, fo]   = sum_fi M^T[fi, e] W^T[fi, fo]   (pre-AR, linearity)
    one fp16 AllReduce over mw (0.5 MB)
    GEMM2: out^T[f, n] = sum_e mw[e, f] A^T[e, n] (+ bias)

All DRAM operands are host-pre-tiled into flat [128, W] row-major blocks so
every load/store is a single large fully-contiguous DMA (per-dma_start issue
cost on the queue engines is ~600 ns, so many small tile DMAs serialize the
whole front of the kernel). Output is produced as flat [128, 4*512] fp16
out^T blocks; the host reassembles and upcasts.
"""

import os
import sys
import types

import numpy as np


def _ensure_axon_hooks_module():
    """bass_utils imports antenv.axon_hooks when tracing; some images
    lack it. Provide a stub (and try to wire the real ctypes hook) so
    trace paths degrade gracefully instead of crashing."""
    try:
        import antenv.axon_hooks  # noqa: F401
        return
    except ImportError:
        pass
    try:
        import antenv
    except ImportError:
        return
    mod = types.ModuleType("antenv.axon_hooks")
    state = {"hook": None}
    mod.get_axon_ntff_profile_hook = lambda: state["hook"]
    mod.set_axon_ntff_profile_hook = lambda h: state.__setitem__("hook", h)
    sys.modules["antenv.axon_hooks"] = mod
    antenv.axon_hooks = mod
    try:
        from trn_agent_boot.trn_boot import _ntff_profile_via_ctypes
        hook = _ntff_profile_via_ctypes("/opt/axon/libaxon_pjrt.so")
        if hook is not None:
            state["hook"] = hook
    except Exception:
        pass


_ensure_axon_hooks_module()

N, E, F = 8192, 1024, 256
P = 128
NC_COUNT = 8
NL = N // NC_COUNT          # 1024 rows per core
NT = NL // P                # 8 row tiles per core
ET = E // P                 # 8 e-chunks of 128
FI = F // P                 # 2 fi-chunks
EH = 512                    # e-half width (one f32 PSUM bank)
NH = 512                    # n-half width for GEMM2 psums

_cache = {}


def _build():
    from concourse import bacc, bass, tile, mybir

    f32 = mybir.dt.float32
    f16 = mybir.dt.float16

    nc = bacc.Bacc("TRN2", target_bir_lowering=False, debug=False,
                   num_devices=NC_COUNT)

    # host-pre-tiled flat operands (see kernel() for the layouts)
    X_d = nc.dram_tensor("X", [P, NT * F], f16, kind="ExternalInput")
    HS_d = nc.dram_tensor("HS", [P, 2 * NT * EH], f16, kind="ExternalInput")
    AT_d = nc.dram_tensor("AT", [P, ET * NL], f16, kind="ExternalInput")
    WT_d = nc.dram_tensor("WT", [P, FI * F], f16, kind="ExternalInput")
    B_d = nc.dram_tensor("bias", [P, FI], f32, kind="ExternalInput")
    out_d = nc.dram_tensor("out", [P, 4 * NH], f16, kind="ExternalOutput")

    with tile.TileContext(nc) as tc:
        with (
            tc.tile_pool(name="const", bufs=1) as constp,
            tc.tile_pool(name="xp", bufs=1) as xp,
            tc.tile_pool(name="hsp", bufs=1) as hsp,
            tc.tile_pool(name="atp", bufs=1) as atp,
            tc.tile_pool(name="mtp", bufs=4) as mtp,
            tc.tile_pool(name="mwp", bufs=1) as mwp,
            tc.tile_pool(name="mrp", bufs=1) as mrp,
            tc.tile_pool(name="outp", bufs=1) as outp,
            tc.tile_pool(name="ps_mt", bufs=2, space="PSUM") as ps_mt,
            tc.tile_pool(name="ps_mw", bufs=2, space="PSUM") as ps_mw,
            tc.tile_pool(name="ps_o", bufs=1, space="PSUM") as ps_o,
            tc.tile_pool(name="dram", bufs=1, space="DRAM") as dramp,
        ):
            # ---- batched loads; x/hs on sync+vector queues (critical
            # path), at on gpsimd, consts on scalar ----
            wt = constp.tile([P, FI * F], f16)
            nc.scalar.dma_start(wt[:], WT_d[:, :])
            bias = constp.tile([P, FI], f32)
            nc.scalar.dma_start(bias[:], B_d[:, :])

            x_all = xp.tile([P, NT * F], f16)
            nc.sync.dma_start(x_all[:], X_d[:, :])
            HHW = NT * EH               # one half of HS, flat width
            hs = []
            for h in range(2):
                t = hsp.tile([P, HHW], f16, name=f"hs{h}")
                eng = nc.sync if h == 0 else nc.scalar
                eng.dma_start(t[:, 0:HHW // 2], HS_d[:, h * HHW:
                                                     h * HHW + HHW // 2])
                eng.dma_start(t[:, HHW // 2:HHW], HS_d[:, h * HHW + HHW // 2:
                                                       (h + 1) * HHW])
                hs.append(t)

            at_all = atp.tile([P, ET * NL], f16)
            half_at = ET * NL // 2
            nc.gpsimd.dma_start(at_all[:, 0:half_at], AT_d[:, 0:half_at])
            nc.gpsimd.dma_start(at_all[:, half_at:], AT_d[:, half_at:])

            # ---- collective bounce buffers ----
            cc_in = dramp.tile([P, ET * F], f16, name="cc_in")
            cc_out = dramp.tile([P, ET * F], f16, name="cc_out",
                                addr_space="Shared")

            # ---- GEMM1 (M^T per e-half) + GEMMW (mw per e-chunk) ----
            mw_all = mwp.tile([P, ET * F], f16)
            for half in range(2):
                mt_sb = []
                for fi in range(FI):
                    ps = ps_mt.tile([P, EH], f32, name="mt_ps")
                    for i in range(NT):
                        nc.tensor.matmul(
                            ps[:],
                            x_all[:, i * F + fi * P: i * F + (fi + 1) * P],
                            hs[half][:, i * EH:(i + 1) * EH],
                            start=(i == 0), stop=(i == NT - 1),
                        )
                    sb = mtp.tile([P, EH], f16, name="mt_sb")
                    if fi == 0:
                        nc.vector.tensor_copy(sb[:], ps[:])
                    else:
                        nc.scalar.copy(sb[:], ps[:])
                    mt_sb.append(sb)
                for jj in range(ET // 2):        # e-chunks within this half
                    j = half * (ET // 2) + jj    # global e-chunk index
                    psw = ps_mw.tile([P, F], f32, name="mw_ps")
                    for fi in range(FI):
                        nc.tensor.matmul(
                            psw[:], mt_sb[fi][:, jj * P:(jj + 1) * P],
                            wt[:, fi * F:(fi + 1) * F],
                            start=(fi == 0), stop=(fi == FI - 1),
                        )
                    dst = mw_all[:, j * F:(j + 1) * F]
                    if jj % 2 == 0:
                        nc.vector.tensor_copy(dst, psw[:])
                    else:
                        nc.scalar.copy(dst, psw[:])

            nc.sync.dma_start(cc_in[:, :], mw_all[:])
            nc.gpsimd.collective_compute(
                "AllReduce",
                mybir.AluOpType.add,
                replica_groups=[list(range(NC_COUNT))],
                ins=[cc_in[:].opt()],
                outs=[cc_out[:].opt()],
            )

            # ---- read back reduced mw; GEMM2 accumulates out^T ----
            mwr = mrp.tile([P, ET * F], f16)
            halfw = ET * F // 2
            nc.sync.dma_start(mwr[:, 0:halfw], cc_out[:, 0:halfw])
            nc.sync.dma_start(mwr[:, halfw:], cc_out[:, halfw:])

            pso = [[ps_o.tile([P, NH], f32, name=f"o_ps{f}{nh}")
                    for nh in range(2)] for f in range(FI)]
            for j in range(ET):
                for f in range(FI):
                    for nh in range(2):
                        nc.tensor.matmul(
                            pso[f][nh][:],
                            mwr[:, j * F + f * P: j * F + (f + 1) * P],
                            at_all[:, j * NL + nh * NH: j * NL + (nh + 1) * NH],
                            start=(j == 0), stop=(j == ET - 1),
                        )
            out_all = outp.tile([P, 4 * NH], f16)
            for f in range(FI):
                for nh in range(2):
                    dst = out_all[:, (f * 2 + nh) * NH:(f * 2 + nh + 1) * NH]
                    nc.vector.tensor_scalar_add(dst, pso[f][nh][:],
                                                bias[:, f:f + 1])
            nc.sync.dma_start(out_d[:, :], out_all[:])

    nc.compile()
    return nc


def _get_nc():
    if "nc" not in _cache:
        _cache["nc"] = _build()
    return _cache["nc"]


def kernel(X, H, W, b):
    from concourse import bass_utils

    nc = _get_nc()

    X = np.asarray(X, dtype=np.float32)
    H = np.asarray(H, dtype=np.float32)
    W = np.asarray(W, dtype=np.float32)
    b = np.asarray(b, dtype=np.float32)

    dv = H.sum(axis=1)
    de = H.sum(axis=0)
    dvis = (1.0 / np.sqrt(dv)).astype(np.float32)
    dei = (1.0 / de).astype(np.float32)

    Hs32 = H * dvis[:, None]
    HS16 = Hs32.astype(np.float16)
    A16 = (Hs32 * dei[None, :]).astype(np.float16)
    X16 = X.astype(np.float16)
    WT16 = np.ascontiguousarray(W.T).astype(np.float16)
    # host tiling: [128, blocks * width] flat layouts (see _build)
    WT_t = np.ascontiguousarray(
        WT16.reshape(FI, P, F).transpose(1, 0, 2).reshape(P, FI * F))
    bias_t = np.ascontiguousarray(
        b.reshape(FI, P).T.astype(np.float32))

    in_maps = []
    for c in range(NC_COUNT):
        sl = slice(c * NL, (c + 1) * NL)
        Xc = X16[sl].reshape(NT, P, F).transpose(1, 0, 2).reshape(P, NT * F)
        HSc = (HS16[sl].reshape(NT, P, 2, EH).transpose(1, 2, 0, 3)
               .reshape(P, 2 * NT * EH))
        ATc = (A16[sl].T.reshape(ET, P, NL).transpose(1, 0, 2)
               .reshape(P, ET * NL))
        in_maps.append({
            "X": np.ascontiguousarray(Xc),
            "HS": np.ascontiguousarray(HSc),
            "AT": np.ascontiguousarray(ATc),
            "WT": WT_t,
            "bias": bias_t,
        })

    res = bass_utils.run_bass_kernel_spmd(
        nc, in_maps, core_ids=list(range(NC_COUNT)),
        trace=bool(int(os.environ.get("HGNN_TRACE", "0"))),
    )
    _cache["last_result"] = res
    shards = []
    for c in range(NC_COUNT):
        o = res.results[c]["out"]             # [128, 4*512] fp16, out^T blocks
        o = o.reshape(P, FI, 2, NH).transpose(2, 3, 1, 0).reshape(NL, F)
        shards.append(o.astype(np.float32))
    return np.ascontiguousarray(np.concatenate(shards, axis=0))
